# revision 1
# baseline (speedup 1.0000x reference)
"""Trainium2 Bass kernel for nn_DataSelectorCGCNN.

Strategy:
  - Host: build the padded/masked per-crystal feature matrix feat [B, D]
    (the ragged gather / data-selector part -- cheap, index-bound), fold the
    b1 bias into an extra ones-row, transpose to featT [DPAD, B], and
    pre-tile W1 into [nN, nK, 128, 512] so every device DMA is a fully
    contiguous block.
  - Device (8 NeuronCores, data-parallel over crystals): each core computes
    h = relu(featT_shard.T @ W1pad) with float32r matmuls (full-rate fp32
    streaming mode on the PE array, ~1.7e-4 absmax accuracy), K accumulated
    in PSUM fp32.
  - Host: scores = h @ (weight_phy*wp + weight_gen*wg)  (0.02% of FLOPs),
    concat shards -> [B, 1] float32.

Device mapping per core:
  lhsT = featT tile [128 K, 128 crystals] (stationary operand, SBUF-resident)
  rhs  = W1   tile [128 K, 512 H]         (moving operand, streamed from HBM)
  out  = PSUM tile [128 crystals, 512 H], accumulated over 47 K-tiles,
         evicted through ScalarE ReLU -> SBUF -> HBM.
"""

import os

import numpy as np

# The axon client in this container has no NTFF profile hook; make sure a
# stray BASS_TRACE in the environment can't route us onto that path.
os.environ.setdefault("BASS_NEVER_TRACE", "1")

import concourse.bacc as bacc
import concourse.mybir as mybir
import concourse.tile as tile
from concourse.bass_utils import run_bass_kernel_spmd

# Problem geometry (hardcoded per contract)
B = 4096
MAX_N = 10
FA = 92
M_NBR = 12
FN = 41
H = 2048
D = MAX_N * (FA + M_NBR * FN + M_NBR + 1)  # 5970
N_CORES = 8
BS = B // N_CORES  # 512 crystals per core
DPAD = 6016  # 47 * 128  (>= D+1; row D carries the ones/bias row)
NK = DPAD // 128  # 47
NMC = BS // 128  # 4
NN = H // 512  # 4

# tuning knobs (selected from on-hardware A/B)
WBUFS = 8
KFUSE = 4
HBUFS = 4

_MM_DT = mybir.dt.float32r
_NP_DT = np.float32

_cache = {}


def _build_nc(reps=1):
    """Build the per-core device program. reps>1 wraps the compute body in a
    hardware loop (used only for timing in test.py)."""
    nc = bacc.Bacc("TRN2", target_bir_lowering=False, debug=False,
                   num_devices=N_CORES)
    ft_d = nc.dram_tensor("featT", [DPAD, BS], _MM_DT, kind="ExternalInput")
    w1_d = nc.dram_tensor("w1t", [NN, NK, 128, 512], _MM_DT,
                          kind="ExternalInput")
    h_d = nc.dram_tensor("hout", [BS, H], mybir.dt.float32,
                         kind="ExternalOutput")

    ftr = ft_d.rearrange("(k p) b -> k p b", p=128)

    with tile.TileContext(nc) as tc:
        with (
            tc.tile_pool(name="ftpool", bufs=1) as ftpool,
            tc.tile_pool(name="wpool", bufs=WBUFS) as wpool,
            tc.tile_pool(name="hpool", bufs=HBUFS) as hpool,
            tc.tile_pool(name="cpool", bufs=1) as cpool,
            tc.tile_pool(name="psum", bufs=2, space="PSUM") as psumpool,
        ):
            zero_bias = cpool.tile([128, 1], mybir.dt.float32)
            nc.any.memset(zero_bias[:], 0.0)

            # featT resident in SBUF: [128, 47, 512] (96.25 KB/partition).
            # Issued on the ACT HWDGE queue so the one-time prologue load
            # runs in parallel with the first W1 pass on the SP queue.
            ft_sb = ftpool.tile([128, NK, BS], _MM_DT)
            for k in range(NK):
                nc.scalar.dma_start(ft_sb[:, k, :], ftr[k])

            def body():
                for n in range(NN):
                    psums = []
                    for mc in range(NMC):
                        pt = psumpool.tile([128, 512], mybir.dt.float32,
                                           name=f"ps{mc}", tag=f"ps{mc}")
                        psums.append(pt)
                    for k0 in range(0, NK, KFUSE):
                        klen = min(KFUSE, NK - k0)
                        wt = wpool.tile([128, KFUSE, 512], _MM_DT,
                                        name="wt", tag="wt")[:, :klen, :]
                        src = w1_d[n, k0:k0 + klen]
                        nc.sync.dma_start(wt[:], src.rearrange("a p c -> p a c"))
                        for j in range(klen):
                            k = k0 + j
                            for mc in range(NMC):
                                nc.tensor.matmul(
                                    psums[mc][:],
                                    ft_sb[:, k, mc * 128:(mc + 1) * 128],
                                    wt[:, j, :],
                                    start=(k == 0),
                                    stop=(k == NK - 1),
                                )
                    for mc in range(NMC):
                        ht = hpool.tile([128, 512], mybir.dt.float32,
                                        name="ht", tag="ht")
                        nc.scalar.activation(
                            ht[:], psums[mc][:],
                            mybir.ActivationFunctionType.Relu,
                            bias=zero_bias[:])
                        nc.sync.dma_start(
                            h_d[mc * 128:(mc + 1) * 128,
                                n * 512:(n + 1) * 512],
                            ht[:])

            if reps > 1:
                with tc.For_i(0, reps, 1):
                    body()
            else:
                body()
    nc.compile()
    return nc


def _host_features(atom_fea, nbr_fea, nbr_fea_idx, starts, lens, max_n):
    """Mirror of the reference gather/pad/concat, producing featT [DPAD, B]
    with a ones row at index D (pairs with the b1 row appended to W1)."""
    N = atom_fea.shape[0]
    max_n = int(max_n)
    ar = np.arange(max_n, dtype=starts.dtype)
    n_use = np.minimum(lens, max_n)
    valid = ar[None, :] < n_use[:, None]                    # [B, max_n]
    pos = np.clip(starts[:, None] + ar[None, :], 0, N - 1)  # [B, max_n]
    mask = valid.astype(np.float32)

    atom_pad = atom_fea[pos] * mask[..., None]              # [B, max_n, FA]
    nbr_pad = (nbr_fea[pos].reshape(B, max_n, M_NBR * FN)
               * mask[..., None])
    nb = nbr_fea_idx[pos] - starts[:, None, None]
    nb = np.maximum(nb, 0)
    nb = np.where(nb >= n_use[:, None, None], 0, nb)
    nb = np.where(valid[..., None], nb, 0)
    idx_feat = nb.astype(np.float32) / max_n
    node_feat = np.concatenate(
        [atom_pad, nbr_pad, idx_feat, mask[..., None]], axis=2)
    feat = node_feat.reshape(B, -1)                         # [B, D]

    featT = np.zeros((DPAD, B), dtype=np.float32)
    featT[:D, :] = feat.T
    featT[D, :] = 1.0  # bias row
    return featT


def _host_w1t(W1, b1):
    """Pad W1 with the b1 bias row, pre-tile to [NN, NK, 128, 512]."""
    w1pad = np.zeros((DPAD, H), dtype=np.float32)
    w1pad[:D, :] = W1
    w1pad[D, :] = b1
    return np.ascontiguousarray(
        w1pad.reshape(NK, 128, NN, 512).transpose(2, 0, 1, 3))


def kernel(atom_fea, nbr_fea, W1, b1, wp, wg, weight_phy, weight_gen,
           nbr_fea_idx, starts, lens, max_n):
    atom_fea = np.asarray(atom_fea, dtype=np.float32)
    nbr_fea = np.asarray(nbr_fea, dtype=np.float32)
    W1 = np.asarray(W1, dtype=np.float32)
    b1 = np.asarray(b1, dtype=np.float32)
    wp = np.asarray(wp, dtype=np.float32).reshape(-1)
    wg = np.asarray(wg, dtype=np.float32).reshape(-1)
    nbr_fea_idx = np.asarray(nbr_fea_idx, dtype=np.int32)
    starts = np.asarray(starts, dtype=np.int32)
    lens = np.asarray(lens, dtype=np.int32)

    assert W1.shape == (D, H) and starts.shape[0] == B

    featT = _host_features(atom_fea, nbr_fea, nbr_fea_idx, starts, lens,
                           max_n)
    w1t = _host_w1t(W1, b1)

    if "nc" not in _cache:
        _cache["nc"] = _build_nc(reps=1)
    nc = _cache["nc"]

    in_maps = [
        {"featT": np.ascontiguousarray(featT[:, c * BS:(c + 1) * BS]),
         "w1t": w1t}
        for c in range(N_CORES)
    ]
    res = run_bass_kernel_spmd(nc, in_maps, core_ids=list(range(N_CORES)))

    wc = (np.float32(weight_phy) * wp
          + np.float32(weight_gen) * wg).astype(np.float32)  # [H]

    scores = np.empty((B, 1), dtype=np.float32)
    for c in range(N_CORES):
        h = res.results[c]["hout"]  # [BS, H] float32
        scores[c * BS:(c + 1) * BS, 0] = h @ wc
    return scores



# revision 4
# speedup vs baseline: 1.0889x; 1.0889x over previous
"""Trainium2 Bass kernel for nn_DataSelectorCGCNN (mixed bf16 / fp8-DoubleRow).

Strategy:
  - Host: build the padded/masked per-crystal feature matrix feat [B, D]
    (ragged gather -- cheap, index-bound), prepend a ones/bias row, and
    split the K=5971 contraction rows into two sections:
      * bf16 section: first 4435 rows (padded to 4480 = 35*128)
      * fp8  section: last 1536 rows (= 6*256), feat scaled by 1/8 and W
        by 8 (product scale 1) so both operands sit in e4m3 normal range
    The fp8 rows run as DoubleRow matmuls (2 K-rows per partition pair,
    half the matmul instructions per K), measured ~2x the bf16 rate on
    hardware.  The fp8 fraction is chosen so the end-to-end rel-err vs
    the fp32 reference stays ~1.8e-2 (< 2e-2 gate); verified in numpy,
    which matches the device bit-near-exactly (host-side casts, exact
    fp8/bf16 products, fp32 PSUM).
  - Device (8 cores, data-parallel over crystals): per core
    z = featT_shard.T @ W  accumulated in PSUM over both sections,
    ScalarE ReLU evicts to SBUF, and a single DVE tensor_tensor_reduce
    per tile applies the combined head vector wc and reduces over H,
    chaining partial sums across H-tiles.  Output is scores [BS, 1]
    directly -- no h writeback to HBM.
  - featT (3.3 MB) + wc stay SBUF-resident (prologue); W (21 MB) streams
    from HBM once per iteration, double-buffered.
"""

import os

import numpy as np
import ml_dtypes

os.environ.setdefault("BASS_NEVER_TRACE", "1")

import concourse.bacc as bacc
import concourse.mybir as mybir
import concourse.tile as tile
from concourse.bass_utils import run_bass_kernel_spmd

# Problem geometry (hardcoded per contract)
B = 4096
MAX_N = 10
FA = 92
M_NBR = 12
FN = 41
H = 2048
D = MAX_N * (FA + M_NBR * FN + M_NBR + 1)  # 5970
KTOT = D + 1          # + ones/bias row
N_CORES = 8
BS = B // N_CORES     # 512 crystals per core
NMC = BS // 128       # 4 crystal blocks
NN = H // 512         # 4 H tiles

# K split: bf16 head, fp8-DoubleRow tail
NC8 = 6               # fp8 chunks of 256 K-rows
K8 = NC8 * 256        # 1536
KBF_REAL = KTOT - K8  # 4435
NKB = (KBF_REAL + 127) // 128  # 35 bf16 chunks (pad to 4480)
KBF = NKB * 128
KFUSE = 5             # bf16 W-chunks per DMA (35 = 7*5)
NKG = NKB // KFUSE    # 7
WBUFS = 4
F8SCALE = 8.0         # feat/8, W*8 in the fp8 section

BF16 = mybir.dt.bfloat16
F8 = mybir.dt.float8e4
NP_BF16 = ml_dtypes.bfloat16
NP_F8 = ml_dtypes.float8_e4m3
DR = mybir.MatmulPerfMode.DoubleRow

_cache = {}


def _build_nc(reps=1):
    """Per-core device program. reps>1 wraps the body in a hardware loop
    (used only for timing in test.py)."""
    nc = bacc.Bacc("TRN2", target_bir_lowering=False, debug=False,
                   num_devices=N_CORES)
    ftb_d = nc.dram_tensor("ftb", [NKB, 128, BS], BF16, kind="ExternalInput")
    ft8_d = nc.dram_tensor("ft8", [NC8, 128, 2, BS], F8, kind="ExternalInput")
    wtb_d = nc.dram_tensor("wtb", [NN, NKG, 128, KFUSE, 512], BF16,
                           kind="ExternalInput")
    wt8_d = nc.dram_tensor("wt8", [NN, 128, NC8, 2, 512], F8,
                           kind="ExternalInput")
    wc_d = nc.dram_tensor("wct", [NN, 128, 512], mybir.dt.float32,
                          kind="ExternalInput")
    score_d = nc.dram_tensor("score", [BS, 1], mybir.dt.float32,
                             kind="ExternalOutput")

    with tile.TileContext(nc) as tc:
        with (
            tc.tile_pool(name="ftpool", bufs=1) as ftpool,
            tc.tile_pool(name="wpool", bufs=WBUFS) as wpool,
            tc.tile_pool(name="w8pool", bufs=2) as w8pool,
            tc.tile_pool(name="hpool", bufs=4) as hpool,
            tc.tile_pool(name="scrpool", bufs=2) as scrpool,
            tc.tile_pool(name="accpool", bufs=2) as accpool,
            tc.tile_pool(name="cpool", bufs=1) as cpool,
            tc.tile_pool(name="psum", bufs=2, space="PSUM") as psumpool,
        ):
            zero_bias = cpool.tile([128, 1], mybir.dt.float32)
            nc.any.memset(zero_bias[:], 0.0)

            # SBUF-resident: featT both sections + wc (one-time prologue)
            ftb_sb = ftpool.tile([128, NKB, BS], BF16)
            for k in range(NKB):
                nc.scalar.dma_start(ftb_sb[:, k], ftb_d[k])
            ft8_sb = ftpool.tile([128, NC8, 2, BS], F8)
            for k in range(NC8):
                nc.scalar.dma_start(ft8_sb[:, k], ft8_d[k])
            wc_sb = cpool.tile([128, NN, 512], mybir.dt.float32)
            for n in range(NN):
                nc.scalar.dma_start(wc_sb[:, n], wc_d[n])

            def body():
                partials = [None] * NMC
                for n in range(NN):
                    psums = [psumpool.tile([128, 512], mybir.dt.float32,
                                           name=f"ps{mc}", tag=f"ps{mc}")
                             for mc in range(NMC)]
                    for kg in range(NKG):
                        wt = wpool.tile([128, KFUSE, 512], BF16,
                                        name="wt", tag="wt")
                        nc.sync.dma_start(
                            wt[:], wtb_d[n, kg].rearrange("p a c -> p a c"))
                        for j in range(KFUSE):
                            k = kg * KFUSE + j
                            for mc in range(NMC):
                                nc.tensor.matmul(
                                    psums[mc][:],
                                    ftb_sb[:, k, mc * 128:(mc + 1) * 128],
                                    wt[:, j, :],
                                    start=(k == 0), stop=False)
                    wt8 = w8pool.tile([128, NC8, 2, 512], F8,
                                      name="wt8", tag="wt8")
                    nc.sync.dma_start(wt8[:], wt8_d[n])
                    for k8 in range(NC8):
                        for mc in range(NMC):
                            nc.tensor.matmul(
                                psums[mc][:],
                                ft8_sb[:, k8, :, mc * 128:(mc + 1) * 128],
                                wt8[:, k8],
                                start=False, stop=(k8 == NC8 - 1),
                                perf_mode=DR)
                    for mc in range(NMC):
                        if n == 0:
                            partials[mc] = accpool.tile(
                                [128, NN], mybir.dt.float32,
                                name=f"red{mc}", tag=f"red{mc}")
                        ht = hpool.tile([128, 512], mybir.dt.float32,
                                        name="ht", tag="ht")
                        nc.scalar.activation(
                            ht[:], psums[mc][:],
                            mybir.ActivationFunctionType.Relu,
                            bias=zero_bias[:])
                        scr = scrpool.tile([128, 512], mybir.dt.float32,
                                           name="scr", tag="scr")
                        nc.vector.tensor_tensor(
                            scr[:], ht[:], wc_sb[:, n],
                            mybir.AluOpType.mult)
                        nc.vector.tensor_reduce(
                            partials[mc][:, n:n + 1], scr[:],
                            axis=mybir.AxisListType.X,
                            op=mybir.AluOpType.add)
                for mc in range(NMC):
                    acc = accpool.tile([128, 1], mybir.dt.float32,
                                       name=f"acc{mc}", tag=f"acc{mc}")
                    nc.vector.tensor_reduce(
                        acc[:], partials[mc][:],
                        axis=mybir.AxisListType.X,
                        op=mybir.AluOpType.add)
                    nc.sync.dma_start(
                        score_d[mc * 128:(mc + 1) * 128, :], acc[:])

            if reps > 1:
                with tc.For_i(0, reps, 1):
                    body()
            else:
                body()
    nc.compile()
    return nc


def _host_features(atom_fea, nbr_fea, nbr_fea_idx, starts, lens, max_n):
    """Mirror of the reference gather/pad/concat, producing featT [D, B]."""
    N = atom_fea.shape[0]
    max_n = int(max_n)
    ar = np.arange(max_n, dtype=starts.dtype)
    n_use = np.minimum(lens, max_n)
    valid = ar[None, :] < n_use[:, None]                    # [B, max_n]
    pos = np.clip(starts[:, None] + ar[None, :], 0, N - 1)  # [B, max_n]
    mask = valid.astype(np.float32)

    atom_pad = atom_fea[pos] * mask[..., None]              # [B, max_n, FA]
    nbr_pad = (nbr_fea[pos].reshape(B, max_n, M_NBR * FN)
               * mask[..., None])
    nb = nbr_fea_idx[pos] - starts[:, None, None]
    nb = np.maximum(nb, 0)
    nb = np.where(nb >= n_use[:, None, None], 0, nb)
    nb = np.where(valid[..., None], nb, 0)
    idx_feat = nb.astype(np.float32) / max_n
    node_feat = np.concatenate(
        [atom_pad, nbr_pad, idx_feat, mask[..., None]], axis=2)
    feat = node_feat.reshape(B, -1)                         # [B, D]
    return np.ascontiguousarray(feat.T)                     # [D, B]


def _host_pack(featT, W1, b1, wc):
    """Split K rows into bf16/fp8 sections and pre-tile for the device.

    Logical rows: row 0 = ones/bias (feat=1, W=b1), rows 1..D = feat/W1.
    Returns full-B arrays; the per-core featT slices are cut in kernel().
    """
    featL = np.concatenate(
        [np.ones((1, B), np.float32), featT], axis=0)       # [KTOT, B]
    WL = np.concatenate([b1[None, :], W1], axis=0)          # [KTOT, H]

    fb = np.zeros((KBF, B), np.float32)
    fb[:KBF_REAL] = featL[:KBF_REAL]
    wb = np.zeros((KBF, H), np.float32)
    wb[:KBF_REAL] = WL[:KBF_REAL]
    f8 = featL[KBF_REAL:] * np.float32(1.0 / F8SCALE)       # [K8, B]
    w8 = WL[KBF_REAL:] * np.float32(F8SCALE)                # [K8, H]

    ftb = np.ascontiguousarray(
        fb.reshape(NKB, 128, B).astype(NP_BF16))
    # DR pair layout: [chunk, p, i, b] = row (chunk*256 + i*128 + p)
    ft8 = np.ascontiguousarray(
        f8.reshape(NC8, 2, 128, B).transpose(0, 2, 1, 3).astype(NP_F8))
    wtb = np.ascontiguousarray(
        wb.astype(NP_BF16)
        .reshape(NKG, KFUSE, 128, NN, 512).transpose(3, 0, 2, 1, 4))
    wt8 = np.ascontiguousarray(
        w8.astype(NP_F8)
        .reshape(NC8, 2, 128, NN, 512).transpose(3, 2, 0, 1, 4))
    wct = np.ascontiguousarray(
        np.broadcast_to(wc.reshape(NN, 1, 512), (NN, 128, 512))
        .astype(np.float32))
    return ftb, ft8, wtb, wt8, wct


def kernel(atom_fea, nbr_fea, W1, b1, wp, wg, weight_phy, weight_gen,
           nbr_fea_idx, starts, lens, max_n):
    atom_fea = np.asarray(atom_fea, dtype=np.float32)
    nbr_fea = np.asarray(nbr_fea, dtype=np.float32)
    W1 = np.asarray(W1, dtype=np.float32)
    b1 = np.asarray(b1, dtype=np.float32)
    wp = np.asarray(wp, dtype=np.float32).reshape(-1)
    wg = np.asarray(wg, dtype=np.float32).reshape(-1)
    nbr_fea_idx = np.asarray(nbr_fea_idx, dtype=np.int32)
    starts = np.asarray(starts, dtype=np.int32)
    lens = np.asarray(lens, dtype=np.int32)

    assert W1.shape == (D, H) and starts.shape[0] == B

    wc = (np.float32(weight_phy) * wp
          + np.float32(weight_gen) * wg).astype(np.float32)  # [H]

    featT = _host_features(atom_fea, nbr_fea, nbr_fea_idx, starts, lens,
                           max_n)
    ftb, ft8, wtb, wt8, wct = _host_pack(featT, W1, b1, wc)

    if "nc" not in _cache:
        _cache["nc"] = _build_nc(reps=1)
    nc = _cache["nc"]

    in_maps = [
        {"ftb": np.ascontiguousarray(ftb[:, :, c * BS:(c + 1) * BS]),
         "ft8": np.ascontiguousarray(ft8[:, :, :, c * BS:(c + 1) * BS]),
         "wtb": wtb, "wt8": wt8, "wct": wct}
        for c in range(N_CORES)
    ]
    res = run_bass_kernel_spmd(nc, in_maps, core_ids=list(range(N_CORES)))

    scores = np.empty((B, 1), dtype=np.float32)
    for c in range(N_CORES):
        scores[c * BS:(c + 1) * BS] = res.results[c]["score"]
    return scores


# revision 7
# speedup vs baseline: 1.1298x; 1.0375x over previous
"""Trainium2 Bass kernel for nn_DataSelectorCGCNN (mixed bf16 / fp8-DoubleRow).

Strategy:
  - Host: build the padded/masked per-crystal feature matrix feat [B, D]
    (ragged gather -- cheap, index-bound), prepend a ones/bias row, and
    split the K=5971 contraction rows into two sections:
      * bf16 section: first 4435 rows (padded to 4480 = 35*128)
      * fp8  section: last 1536 rows (= 6*256), feat scaled by 1/8 and W
        by 8 (product scale 1) so both operands sit in e4m3 normal range
    The fp8 rows run as DoubleRow matmuls (2 K-rows per partition pair,
    half the matmul instructions per K), measured ~2x the bf16 rate on
    hardware.  The fp8 fraction is chosen so the end-to-end rel-err vs
    the fp32 reference stays ~1.8e-2 (< 2e-2 gate); verified in numpy,
    which matches the device bit-near-exactly (host-side casts, exact
    fp8/bf16 products, fp32 PSUM).
  - Device (8 cores, data-parallel over crystals): per core
    z = featT_shard.T @ W  accumulated in PSUM over both sections,
    ScalarE ReLU evicts to SBUF, and a single DVE tensor_tensor_reduce
    per tile applies the combined head vector wc and reduces over H,
    chaining partial sums across H-tiles.  Output is scores [BS, 1]
    directly -- no h writeback to HBM.
  - featT (3.3 MB) + wc stay SBUF-resident (prologue); W (21 MB) streams
    from HBM once per iteration, double-buffered.
"""

import os

import numpy as np
import ml_dtypes

os.environ.setdefault("BASS_NEVER_TRACE", "1")

import concourse.bacc as bacc
import concourse.mybir as mybir
import concourse.tile as tile
from concourse.bass_utils import run_bass_kernel_spmd

# Problem geometry (hardcoded per contract)
B = 4096
MAX_N = 10
FA = 92
M_NBR = 12
FN = 41
H = 2048
D = MAX_N * (FA + M_NBR * FN + M_NBR + 1)  # 5970
KTOT = D + 1          # + ones/bias row
N_CORES = 8
BS = B // N_CORES     # 512 crystals per core
NMC = BS // 128       # 4 crystal blocks
NN = H // 512         # 4 H tiles

# K split: bf16 head, fp8-DoubleRow tail
NC8 = 6               # fp8 chunks of 256 K-rows
K8 = NC8 * 256        # 1536
KBF_REAL = KTOT - K8  # 4435
NKB = (KBF_REAL + 127) // 128  # 35 bf16 chunks (pad to 4480)
KBF = NKB * 128
KFUSE = 5             # bf16 W-chunks per DMA (35 = 7*5)
NKG = NKB // KFUSE    # 7
WBUFS = 4
F8SCALE = 8.0         # feat/8, W*8 in the fp8 section

BF16 = mybir.dt.bfloat16
F8 = mybir.dt.float8e4
NP_BF16 = ml_dtypes.bfloat16
NP_F8 = ml_dtypes.float8_e4m3
DR = mybir.MatmulPerfMode.DoubleRow

_cache = {}


def _build_nc(reps=1):
    """Per-core device program. reps>1 wraps the body in a hardware loop
    (used only for timing in test.py)."""
    nc = bacc.Bacc("TRN2", target_bir_lowering=False, debug=False,
                   num_devices=N_CORES)
    ftb_d = nc.dram_tensor("ftb", [NKB, 128, BS], BF16, kind="ExternalInput")
    ft8_d = nc.dram_tensor("ft8", [NC8, 128, 2, BS], F8, kind="ExternalInput")
    wtb_d = nc.dram_tensor("wtb", [NN, NKG, 128, KFUSE, 512], BF16,
                           kind="ExternalInput")
    wt8_d = nc.dram_tensor("wt8", [NN, 128, NC8, 2, 512], F8,
                           kind="ExternalInput")
    wc_d = nc.dram_tensor("wct", [NN, 128, 512], mybir.dt.float32,
                          kind="ExternalInput")
    score_d = nc.dram_tensor("score", [BS, 1], mybir.dt.float32,
                             kind="ExternalOutput")

    with tile.TileContext(nc) as tc:
        with (
            tc.tile_pool(name="ftpool", bufs=1) as ftpool,
            tc.tile_pool(name="wpool", bufs=WBUFS) as wpool,
            tc.tile_pool(name="w8pool", bufs=2) as w8pool,
            tc.tile_pool(name="scrpool", bufs=2) as scrpool,
            tc.tile_pool(name="accpool", bufs=2) as accpool,
            tc.tile_pool(name="cpool", bufs=1) as cpool,
            tc.tile_pool(name="psum", bufs=2, space="PSUM") as psumpool,
        ):
            # SBUF-resident: featT both sections + wc (one-time prologue)
            ftb_sb = ftpool.tile([128, NKB, BS], BF16)
            for k in range(NKB):
                nc.scalar.dma_start(ftb_sb[:, k], ftb_d[k])
            ft8_sb = ftpool.tile([128, NC8, 2, BS], F8)
            for k in range(NC8):
                nc.scalar.dma_start(ft8_sb[:, k], ft8_d[k])
            wc_sb = cpool.tile([128, NN, 512], mybir.dt.float32)
            for n in range(NN):
                nc.scalar.dma_start(wc_sb[:, n], wc_d[n])

            def body():
                partials = [None] * NMC
                for n in range(NN):
                    psums = [psumpool.tile([128, 512], mybir.dt.float32,
                                           name=f"ps{mc}", tag=f"ps{mc}")
                             for mc in range(NMC)]
                    for kg in range(NKG):
                        wt = wpool.tile([128, KFUSE, 512], BF16,
                                        name="wt", tag="wt")
                        nc.sync.dma_start(
                            wt[:], wtb_d[n, kg].rearrange("p a c -> p a c"))
                        for j in range(KFUSE):
                            k = kg * KFUSE + j
                            for mc in range(NMC):
                                nc.tensor.matmul(
                                    psums[mc][:],
                                    ftb_sb[:, k, mc * 128:(mc + 1) * 128],
                                    wt[:, j, :],
                                    start=(k == 0), stop=False)
                    wt8 = w8pool.tile([128, NC8, 2, 512], F8,
                                      name="wt8", tag="wt8")
                    nc.sync.dma_start(wt8[:], wt8_d[n])
                    for k8 in range(NC8):
                        for mc in range(NMC):
                            nc.tensor.matmul(
                                psums[mc][:],
                                ft8_sb[:, k8, :, mc * 128:(mc + 1) * 128],
                                wt8[:, k8],
                                start=False, stop=(k8 == NC8 - 1),
                                perf_mode=DR)
                    for mc in range(NMC):
                        if n == 0:
                            partials[mc] = accpool.tile(
                                [128, NN], mybir.dt.float32,
                                name=f"red{mc}", tag=f"red{mc}")
                        # fused relu + wc-mult + H-reduce, straight from PSUM
                        scr = scrpool.tile([128, 512], mybir.dt.float32,
                                           name="scr", tag="scr")
                        nc.vector.scalar_tensor_tensor(
                            out=scr[:], in0=psums[mc][:], scalar=0.0,
                            in1=wc_sb[:, n],
                            op0=mybir.AluOpType.max,
                            op1=mybir.AluOpType.mult,
                            accum_out=partials[mc][:, n:n + 1])
                for mc in range(NMC):
                    acc = accpool.tile([128, 1], mybir.dt.float32,
                                       name=f"acc{mc}", tag=f"acc{mc}")
                    nc.vector.tensor_reduce(
                        acc[:], partials[mc][:],
                        axis=mybir.AxisListType.X,
                        op=mybir.AluOpType.add)
                    nc.sync.dma_start(
                        score_d[mc * 128:(mc + 1) * 128, :], acc[:])

            if reps > 1:
                with tc.For_i(0, reps, 1):
                    body()
            else:
                body()
    nc.compile()
    return nc


def _host_features(atom_fea, nbr_fea, nbr_fea_idx, starts, lens, max_n):
    """Mirror of the reference gather/pad/concat, producing featT [D, B]."""
    N = atom_fea.shape[0]
    max_n = int(max_n)
    ar = np.arange(max_n, dtype=starts.dtype)
    n_use = np.minimum(lens, max_n)
    valid = ar[None, :] < n_use[:, None]                    # [B, max_n]
    pos = np.clip(starts[:, None] + ar[None, :], 0, N - 1)  # [B, max_n]
    mask = valid.astype(np.float32)

    atom_pad = atom_fea[pos] * mask[..., None]              # [B, max_n, FA]
    nbr_pad = (nbr_fea[pos].reshape(B, max_n, M_NBR * FN)
               * mask[..., None])
    nb = nbr_fea_idx[pos] - starts[:, None, None]
    nb = np.maximum(nb, 0)
    nb = np.where(nb >= n_use[:, None, None], 0, nb)
    nb = np.where(valid[..., None], nb, 0)
    idx_feat = nb.astype(np.float32) / max_n
    node_feat = np.concatenate(
        [atom_pad, nbr_pad, idx_feat, mask[..., None]], axis=2)
    feat = node_feat.reshape(B, -1)                         # [B, D]
    return np.ascontiguousarray(feat.T)                     # [D, B]


def _host_pack(featT, W1, b1, wc):
    """Split K rows into bf16/fp8 sections and pre-tile for the device.

    Logical rows: row 0 = ones/bias (feat=1, W=b1), rows 1..D = feat/W1.
    Returns full-B arrays; the per-core featT slices are cut in kernel().
    """
    featL = np.concatenate(
        [np.ones((1, B), np.float32), featT], axis=0)       # [KTOT, B]
    WL = np.concatenate([b1[None, :], W1], axis=0)          # [KTOT, H]

    fb = np.zeros((KBF, B), np.float32)
    fb[:KBF_REAL] = featL[:KBF_REAL]
    wb = np.zeros((KBF, H), np.float32)
    wb[:KBF_REAL] = WL[:KBF_REAL]
    f8 = featL[KBF_REAL:] * np.float32(1.0 / F8SCALE)       # [K8, B]
    w8 = WL[KBF_REAL:] * np.float32(F8SCALE)                # [K8, H]

    ftb = np.ascontiguousarray(
        fb.reshape(NKB, 128, B).astype(NP_BF16))
    # DR pair layout: [chunk, p, i, b] = row (chunk*256 + i*128 + p)
    ft8 = np.ascontiguousarray(
        f8.reshape(NC8, 2, 128, B).transpose(0, 2, 1, 3).astype(NP_F8))
    wtb = np.ascontiguousarray(
        wb.astype(NP_BF16)
        .reshape(NKG, KFUSE, 128, NN, 512).transpose(3, 0, 2, 1, 4))
    wt8 = np.ascontiguousarray(
        w8.astype(NP_F8)
        .reshape(NC8, 2, 128, NN, 512).transpose(3, 2, 0, 1, 4))
    wct = np.ascontiguousarray(
        np.broadcast_to(wc.reshape(NN, 1, 512), (NN, 128, 512))
        .astype(np.float32))
    return ftb, ft8, wtb, wt8, wct


def kernel(atom_fea, nbr_fea, W1, b1, wp, wg, weight_phy, weight_gen,
           nbr_fea_idx, starts, lens, max_n):
    atom_fea = np.asarray(atom_fea, dtype=np.float32)
    nbr_fea = np.asarray(nbr_fea, dtype=np.float32)
    W1 = np.asarray(W1, dtype=np.float32)
    b1 = np.asarray(b1, dtype=np.float32)
    wp = np.asarray(wp, dtype=np.float32).reshape(-1)
    wg = np.asarray(wg, dtype=np.float32).reshape(-1)
    nbr_fea_idx = np.asarray(nbr_fea_idx, dtype=np.int32)
    starts = np.asarray(starts, dtype=np.int32)
    lens = np.asarray(lens, dtype=np.int32)

    assert W1.shape == (D, H) and starts.shape[0] == B

    wc = (np.float32(weight_phy) * wp
          + np.float32(weight_gen) * wg).astype(np.float32)  # [H]

    featT = _host_features(atom_fea, nbr_fea, nbr_fea_idx, starts, lens,
                           max_n)
    ftb, ft8, wtb, wt8, wct = _host_pack(featT, W1, b1, wc)

    if "nc" not in _cache:
        _cache["nc"] = _build_nc(reps=1)
    nc = _cache["nc"]

    in_maps = [
        {"ftb": np.ascontiguousarray(ftb[:, :, c * BS:(c + 1) * BS]),
         "ft8": np.ascontiguousarray(ft8[:, :, :, c * BS:(c + 1) * BS]),
         "wtb": wtb, "wt8": wt8, "wct": wct}
        for c in range(N_CORES)
    ]
    res = run_bass_kernel_spmd(nc, in_maps, core_ids=list(range(N_CORES)))

    scores = np.empty((B, 1), dtype=np.float32)
    for c in range(N_CORES):
        scores[c * BS:(c + 1) * BS] = res.results[c]["score"]
    return scores
